# revision 19
# baseline (speedup 1.0000x reference)
"""Trainium2 Bass kernel for nn_Attention_5042291605734.

Full multi-head attention (B=1, S=4096, DIM=1024, H=16, HD=64), fp32.

Sharding: 2 heads per core (tensor-parallel on inner dim). Each core
computes q/k/v projections for its 128-wide inner slice, full softmax
attention for its 2 heads, and a partial output projection against its
128-row slice of Wo. The host sums the 8 partial outputs and adds the
bias terms (bv @ Wo + bo), which are exactly linear corrections.

Per-core dataflow (all layouts chosen so no on-chip transposes of big
tensors are needed; host passes x pre-transposed):
  xT [DIM, S] (host-transposed)  --mm-->  QT/KT/VT [128, S] (inner on
  partitions).  VT is PE-transposed per 128-tile into V natural [k, hd]
  with an appended ones column (so attn@V also produces the softmax
  denominator as row 64 of the PSUM accumulator).
  Scores are computed in [k, q] layout: ST_tile = KT_slice.T @ QT.
  exp() runs on the Scalar engine PSUM->SBUF; no max-subtraction is
  needed (|scores| <= ~16, exp stays well inside fp32 range).
  U^T[hd(+den), q] += V_aug.T @ exp(ST) accumulated over k tiles.
  Normalization multiplies U^T rows by a PE-broadcast of 1/den.
  Output projection: out[s, :] = O^T[:, s_tile].T @ Wo_c.
"""

import os
from contextlib import ExitStack

import numpy as np

import concourse.bass as bass
import concourse.mybir as mybir
import concourse.tile as tile
from concourse.bass_utils import run_bass_kernel_spmd
from concourse.vector_clock import ScopedClock as _ScopedClock, VectorClock as _VectorClock

_N_PROCS = 27


def _drain_and_barrier_split(self, tick_clock, wait_clock):
    """Replacement for TileContext._drain_and_barrier: the stock version puts
    every live proc's final tick on ONE drain instruction, and this walrus
    build rejects instructions with >2 sync-wait commands (CoreV3GenImpl
    setupSyncWait "Too many sync wait commands"). Split the global-clock wait
    across several sequential drains, one proc each."""
    gc = tick_clock.global_clock
    active = [p for p in range(_N_PROCS) if gc[p] > 0]
    groups = [[p] for p in active] or [[]]
    for grp in groups:
        di = self.nc.sync.drain()
        vc = _VectorClock([gc[p] if p in grp else 0 for p in range(_N_PROCS)])
        wait_clock.add_sem_waits(di.ins, _ScopedClock({None: vc}))
    self.nc.all_engine_barrier()
    assert self.sems is not None
    popped = self.nc._tile_sem_poison_stack.pop()
    assert popped is self._sem_poison
    self.nc.clear_and_free_semaphores(list(self.sems.allocated().values()))
    self.nc.all_engine_barrier()


tile.TileContext._drain_and_barrier = _drain_and_barrier_split


_DMA_INST_TYPES = ("InstDMACopy", "InstDMA", "InstDmaTransposeAnt", "InstDMAGatherAnt",
                   "InstTriggeredCopy", "InstCollectiveCompute")


def _split_multi_waits(nc):
    """This walrus build encodes at most ONE sync-wait command per TPB
    instruction (CoreV3GenImpl setupSyncWait fails with "Too many sync wait
    commands" otherwise). Tile attaches several waits to one instruction, so
    hoist all but one wait onto no-op instructions inserted just before, on
    the same engine. For DMAs this gates the descriptor issue on the issuing
    engine instead of the queue, which is more conservative but correct."""
    for f in nc.m.functions:
        for bb in f.blocks:
            insts = bb.instructions
            out = []
            changed = False
            for inst in insts:
                si = inst.sync_info
                if (si is not None and si.on_wait and len(si.on_wait) > 1
                        and inst.engine != mybir.EngineType.Unassigned):
                    waits = list(si.on_wait)
                    for w in waits[:-1]:
                        out.append(mybir.InstNoOp(
                            name=nc.get_next_instruction_name(),
                            engine=inst.engine,
                            sync_info=mybir.SyncInfo(on_wait=[w], on_update=[]),
                            bass_nofuse=True,
                        ))
                    inst.sync_info = mybir.SyncInfo(
                        on_wait=[waits[-1]], on_update=si.on_update)
                    changed = True
                out.append(inst)
            if changed:
                bb.instructions = out


S = 4096
DIM = 1024
INNER = 1024
H = 16
HD = 64
N_CORES = 8
ICPC = INNER // N_CORES  # inner dims per core = 128

F32 = mybir.dt.float32
F32R = mybir.dt.float32r
EXP = mybir.ActivationFunctionType.Exp

LAST_RESULT = None  # BassKernelResults from the most recent run (for test.py)


def _build_nc() -> bass.Bass:
    nc = bass.Bass()

    xT = nc.declare_dram_parameter("xT", [DIM, S], F32R, isOutput=False)
    wq = nc.declare_dram_parameter("wq", [DIM, ICPC], F32R, isOutput=False)
    wk = nc.declare_dram_parameter("wk", [DIM, ICPC], F32R, isOutput=False)
    wv = nc.declare_dram_parameter("wv", [DIM, ICPC], F32R, isOutput=False)
    wo0 = nc.declare_dram_parameter("wo0", [HD, DIM], F32R, isOutput=False)
    wo1 = nc.declare_dram_parameter("wo1", [HD, DIM], F32R, isOutput=False)
    bq = nc.declare_dram_parameter("bq", [ICPC, 1], F32, isOutput=False)
    bk = nc.declare_dram_parameter("bk", [ICPC, 1], F32, isOutput=False)
    ident = nc.declare_dram_parameter("ident", [128, 128], F32R, isOutput=False)
    onesv = nc.declare_dram_parameter("onesv", [128, 128], F32R, isOutput=False)
    out = nc.declare_dram_parameter("out", [S, DIM], F32, isOutput=True)

    def mm(out_ap, lhsT, rhs, **kw):
        nc.tensor.matmul(out_ap, lhsT, rhs, **kw)

    with tile.TileContext(nc) as tc, ExitStack() as ctx:
        xp = ctx.enter_context(tc.tile_pool(name="xp", bufs=2))
        wp = ctx.enter_context(tc.tile_pool(name="wp", bufs=1))
        bigp = ctx.enter_context(tc.tile_pool(name="bigp", bufs=1))
        vtp = ctx.enter_context(tc.tile_pool(name="vtp", bufs=2))
        exp_pool = ctx.enter_context(tc.tile_pool(name="exp_pool", bufs=4))
        rbp = ctx.enter_context(tc.tile_pool(name="rbp", bufs=2))
        denp = ctx.enter_context(tc.tile_pool(name="denp", bufs=2))
        outp = ctx.enter_context(tc.tile_pool(name="outp", bufs=3))
        psA = ctx.enter_context(tc.tile_pool(name="psA", bufs=2, space="PSUM"))
        psU = ctx.enter_context(tc.tile_pool(name="psU", bufs=1, space="PSUM"))
        psM = ctx.enter_context(tc.tile_pool(name="psM", bufs=2, space="PSUM"))

        # --- weights / constants into SBUF ---
        wq_sb = wp.tile([128, 8 * 128], F32R, tag="wq", name="wq_sb")
        wk_sb = wp.tile([128, 8 * 128], F32R, tag="wk", name="wk_sb")
        wv_sb = wp.tile([128, 8 * 128], F32R, tag="wv", name="wv_sb")
        wo0_sb = wp.tile([64, DIM], F32R, tag="wo0", name="wo0_sb")
        wo1_sb = wp.tile([64, DIM], F32R, tag="wo1", name="wo1_sb")
        bq_sb = wp.tile([128, 1], F32, tag="bq", name="bq_sb")
        bk_sb = wp.tile([128, 1], F32, tag="bk", name="bk_sb")
        id_sb = wp.tile([128, 128], F32R, tag="id", name="id_sb")
        ones_sb = wp.tile([128, 128], F32R, tag="ones", name="ones_sb")

        # [DIM, 128] -> [128, dc*128+m] with element (p, dc*128+m) = W[dc*128+p, m]
        nc.sync.dma_start(wq_sb.rearrange("p (c m) -> p c m", c=8),
                          wq.rearrange("(c p) m -> p c m", p=128))
        nc.sync.dma_start(wk_sb.rearrange("p (c m) -> p c m", c=8),
                          wk.rearrange("(c p) m -> p c m", p=128))
        nc.sync.dma_start(wv_sb.rearrange("p (c m) -> p c m", c=8),
                          wv.rearrange("(c p) m -> p c m", p=128))
        nc.sync.dma_start(wo0_sb[:], wo0[:, :])
        nc.sync.dma_start(wo1_sb[:], wo1[:, :])
        nc.sync.dma_start(bq_sb[:], bq[:, :])
        nc.sync.dma_start(bk_sb[:], bk[:, :])
        nc.sync.dma_start(id_sb[:], ident[:, :])
        nc.sync.dma_start(ones_sb[:], onesv[:, :])

        # --- persistent big SBUF tensors ---
        qt_sb = bigp.tile([128, S], F32R, tag="qt", name="qt_sb")
        kt_sb = bigp.tile([128, S], F32R, tag="kt", name="kt_sb")
        ot0_sb = bigp.tile([64, S], F32R, tag="ot0", name="ot0_sb")
        ot1_sb = bigp.tile([64, S], F32R, tag="ot1", name="ot1_sb")
        # V natural tiles: per k-tile 130 cols = [V_h0(64) | ones | V_h1(64) | ones]
        v_sb = bigp.tile([128, 32 * 130], F32R, tag="v", name="v_sb")

        # ones columns of v_sb (cols 64 + 130*t and 129 + 130*t); memset is
        # ISA-invalid for f32r, so copy from the all-ones constant instead
        for t in range(32):
            nc.vector.tensor_copy(v_sb[:, t * 130 + 64 : t * 130 + 65], ones_sb[:, 0:1])
            nc.vector.tensor_copy(v_sb[:, t * 130 + 129 : t * 130 + 130], ones_sb[:, 0:1])


        # ---------------- Phase A: projections ----------------
        # For each 512-wide s-span: load xT[:, span] (all 8 dim-chunks),
        # compute QT/KT/VT [128, 512], transpose VT into V natural tiles.
        for s8 in range(8):
            x8 = xp.tile([128, 8 * 512], F32R, tag="x8", name="x8")
            for dc in range(8):
                nc.sync.dma_start(
                    x8[:, dc * 512 : (dc + 1) * 512],
                    xT[dc * 128 : (dc + 1) * 128, s8 * 512 : (s8 + 1) * 512],
                )
            span = slice(s8 * 512, (s8 + 1) * 512)

            ps_q = psA.tile([128, 512], F32, tag="sc", name="ps_q")
            for dc in range(8):
                mm(ps_q, wq_sb[:, dc * 128 : (dc + 1) * 128],
                   x8[:, dc * 512 : (dc + 1) * 512],
                   start=(dc == 0), stop=(dc == 7))
            nc.vector.tensor_scalar_add(qt_sb[:, span], ps_q, bq_sb[:, 0:1])

            ps_k = psA.tile([128, 512], F32, tag="sc", name="ps_k")
            for dc in range(8):
                mm(ps_k, wk_sb[:, dc * 128 : (dc + 1) * 128],
                   x8[:, dc * 512 : (dc + 1) * 512],
                   start=(dc == 0), stop=(dc == 7))
            nc.vector.tensor_scalar_add(kt_sb[:, span], ps_k, bk_sb[:, 0:1])

            ps_v = psA.tile([128, 512], F32, tag="sc", name="ps_v")
            for dc in range(8):
                mm(ps_v, wv_sb[:, dc * 128 : (dc + 1) * 128],
                   x8[:, dc * 512 : (dc + 1) * 512],
                   start=(dc == 0), stop=(dc == 7))
            vt8 = vtp.tile([128, 512], F32R, tag="vt8", name="vt8")
            nc.vector.tensor_copy(vt8[:], ps_v)

            # transpose VT -> V natural [k, inner] per 128-wide k tile
            for kti in range(4):
                kt = s8 * 4 + kti
                pt = psM.tile([128, 128], F32R, tag="m", name="pt")
                nc.tensor.transpose(pt, vt8[:, kti * 128 : (kti + 1) * 128], id_sb[:])
                base = kt * 130
                nc.vector.tensor_copy(v_sb[:, base : base + 64], pt[:, 0:64])
                nc.vector.tensor_copy(v_sb[:, base + 65 : base + 129], pt[:, 64:128])

        # ---------------- Phase B: attention ----------------
        # The closeout of q-chunk qc is software-pipelined: the PSUM-freeing
        # eviction (DVE only) runs right after qc's kt-loop, but the PE-using
        # closeout (1/den broadcast, normalize, output projection) is emitted
        # AFTER qc+1's kt-loop so those matmuls never stall the PE stream at
        # the qc boundary (which re-throttled HAM to 1.2 GHz for ~50us each).
        def closeout2(qc, rdens):
            qspan = slice(qc * 512, (qc + 1) * 512)
            for h, ot_h in ((0, ot0_sb), (1, ot1_sb)):
                rb_ps = psM.tile([128, 512], F32, tag="m", name="rb_ps")
                mm(rb_ps, ones_sb[64:65, :], rdens[h][64:65, :],
                   start=True, stop=True)
                rb = rbp.tile([128, 512], F32, tag="rb", name="rb")
                nc.vector.tensor_copy(rb[:], rb_ps)
                nc.vector.tensor_mul(ot_h[:, qspan], ot_h[:, qspan],
                                     rb[0:64, :])
            for sti in range(4):
                st = qc * 4 + sti
                for half in (0, 1):
                    z = psM.tile([128, 512], F32, tag="m", name="z")
                    mm(z, ot0_sb[:, st * 128 : (st + 1) * 128],
                       wo0_sb[:, half * 512 : (half + 1) * 512],
                       start=True, stop=False)
                    mm(z, ot1_sb[:, st * 128 : (st + 1) * 128],
                       wo1_sb[:, half * 512 : (half + 1) * 512],
                       start=False, stop=True)
                    ob = outp.tile([128, 512], F32, tag="ob", name="ob")
                    nc.vector.tensor_copy(ob[:], z)
                    nc.sync.dma_start(
                        out[st * 128 : (st + 1) * 128,
                            half * 512 : (half + 1) * 512],
                        ob[:],
                    )

        prev_closeout = None
        for qc in range(8):
            qspan = slice(qc * 512, (qc + 1) * 512)
            u0 = psU.tile([65, 512], F32, tag="u0", name="u0")
            u1 = psU.tile([65, 512], F32, tag="u1", name="u1")
            for kt in range(32):
                ps = psA.tile([128, 1024], F32, tag="sc", name="ps_s")
                # scores^T [k, q] for both heads (row groups 0:64 / 64:128)
                mm(ps[:, 0:512],
                   kt_sb[0:64, kt * 128 : (kt + 1) * 128],
                   qt_sb[0:64, qspan], start=True, stop=True)
                mm(ps[:, 512:1024],
                   kt_sb[64:128, kt * 128 : (kt + 1) * 128],
                   qt_sb[64:128, qspan], start=True, stop=True)
                ex = exp_pool.tile([128, 1024], F32R, tag="ex", name="ex")
                nc.scalar.activation(ex[:], ps[:], EXP)
                base = kt * 130
                mm(u0, v_sb[:, base : base + 65], ex[:, 0:512],
                   start=(kt == 0), stop=(kt == 31))
                mm(u1, v_sb[:, base + 65 : base + 130], ex[:, 512:1024],
                   start=(kt == 0), stop=(kt == 31))

            # Previous q-chunk's PE-side closeout goes FIRST in the engine
            # streams at this boundary: its deps completed during our kt-loop,
            # so it bridges the PE (and DVE) pipeline bubble while this
            # chunk's eviction below waits for the final attn@V matmuls.
            if prev_closeout is not None:
                prev_closeout()
                prev_closeout = None

            # prompt eviction (DVE only; frees the U PSUM tiles). The slow
            # single-lane reciprocal must NOT read PSUM directly - that holds
            # the U tile slot for ~3.3us and stalls the next q-chunk's attn@V.
            rdens = {}
            for h, u_h, ot_h in ((0, u0, ot0_sb), (1, u1, ot1_sb)):
                nc.vector.tensor_copy(ot_h[:, qspan], u_h[0:64, :])
                # denominator row stays at partition 64 end-to-end
                den = denp.tile([65, 512], F32, tag="den", name="den")
                nc.vector.tensor_copy(den[64:65, :], u_h[64:65, :])
                den2 = denp.tile([65, 512], F32, tag="den2", name="den2")
                nc.vector.reciprocal(den2[64:65, :], den[64:65, :])
                rden = denp.tile([65, 512], F32R, tag="rden", name="rden", bufs=4)
                nc.vector.tensor_copy(rden[64:65, :], den2[64:65, :])
                rdens[h] = rden

            prev_closeout = (lambda q, r: lambda: closeout2(q, r))(qc, rdens)
        prev_closeout()

    _split_multi_waits(nc)
    return nc


_NC_CACHE: dict = {}


def kernel(x, Wq, bq, Wk, bk, Wv, bv, Wo, bo):
    global LAST_RESULT
    x = np.asarray(x, dtype=np.float32)
    Wq = np.asarray(Wq, dtype=np.float32)
    Wk = np.asarray(Wk, dtype=np.float32)
    Wv = np.asarray(Wv, dtype=np.float32)
    Wo = np.asarray(Wo, dtype=np.float32)
    bq = np.asarray(bq, dtype=np.float32)
    bk = np.asarray(bk, dtype=np.float32)
    bv = np.asarray(bv, dtype=np.float32)
    bo = np.asarray(bo, dtype=np.float32)

    if "nc" not in _NC_CACHE:
        _NC_CACHE["nc"] = _build_nc()
    nc = _NC_CACHE["nc"]

    xT = np.ascontiguousarray(x.reshape(S, DIM).T)
    ident = np.eye(128, dtype=np.float32)
    onesv = np.ones((128, 128), dtype=np.float32)

    in_maps = []
    for c in range(N_CORES):
        sl = slice(c * ICPC, (c + 1) * ICPC)
        in_maps.append({
            "xT": xT,
            "wq": np.ascontiguousarray(Wq[:, sl]),
            "wk": np.ascontiguousarray(Wk[:, sl]),
            "wv": np.ascontiguousarray(Wv[:, sl]),
            "wo0": np.ascontiguousarray(Wo[c * ICPC : c * ICPC + HD, :]),
            "wo1": np.ascontiguousarray(Wo[c * ICPC + HD : (c + 1) * ICPC, :]),
            "bq": np.ascontiguousarray(bq[sl].reshape(ICPC, 1)),
            "bk": np.ascontiguousarray(bk[sl].reshape(ICPC, 1)),
            "ident": ident,
            "onesv": onesv,
        })

    res = run_bass_kernel_spmd(
        nc, in_maps, core_ids=list(range(N_CORES)),
        trace=bool(int(os.environ.get("KERNEL_TRACE", "0"))),
    )
    LAST_RESULT = res

    acc = np.zeros((S, DIM), dtype=np.float64)
    for r in res.results:
        acc += r["out"].astype(np.float64)
    # exact linear corrections handled on host: v-bias and output bias
    acc += (bv.astype(np.float64) @ Wo.astype(np.float64))[None, :]
    acc += bo.astype(np.float64)[None, :]
    return acc.astype(np.float32).reshape(1, S, DIM)


# revision 20
# speedup vs baseline: 1.1996x; 1.1996x over previous
"""Trainium2 Bass kernel for nn_Attention_5042291605734.

Full multi-head attention (B=1, S=4096, DIM=1024, H=16, HD=64), fp32.

Sharding: 2 heads per core (tensor-parallel on inner dim). Each core
computes q/k/v projections for its 128-wide inner slice, full softmax
attention for its 2 heads, and a partial output projection against its
128-row slice of Wo. The host sums the 8 partial outputs and adds the
bias terms (bv @ Wo + bo), which are exactly linear corrections.

Per-core dataflow (all layouts chosen so no on-chip transposes of big
tensors are needed; host passes x pre-transposed):
  xT [DIM, S] (host-transposed)  --mm-->  QT/KT/VT [128, S] (inner on
  partitions).  VT is PE-transposed per 128-tile into V natural [k, hd]
  with an appended ones column (so attn@V also produces the softmax
  denominator as row 64 of the PSUM accumulator).
  Scores are computed in [k, q] layout: ST_tile = KT_slice.T @ QT.
  exp() runs on the Scalar engine PSUM->SBUF; no max-subtraction is
  needed (|scores| <= ~16, exp stays well inside fp32 range).
  U^T[hd(+den), q] += V_aug.T @ exp(ST) accumulated over k tiles.
  Normalization multiplies U^T rows by a PE-broadcast of 1/den.
  Output projection: out[s, :] = O^T[:, s_tile].T @ Wo_c.
"""

import os
from contextlib import ExitStack

import numpy as np

import concourse.bass as bass
import concourse.mybir as mybir
import concourse.tile as tile
from concourse.bass_utils import run_bass_kernel_spmd
from concourse.vector_clock import ScopedClock as _ScopedClock, VectorClock as _VectorClock

_N_PROCS = 27


def _drain_and_barrier_split(self, tick_clock, wait_clock):
    """Replacement for TileContext._drain_and_barrier: the stock version puts
    every live proc's final tick on ONE drain instruction, and this walrus
    build rejects instructions with >2 sync-wait commands (CoreV3GenImpl
    setupSyncWait "Too many sync wait commands"). Split the global-clock wait
    across several sequential drains, one proc each."""
    gc = tick_clock.global_clock
    active = [p for p in range(_N_PROCS) if gc[p] > 0]
    groups = [[p] for p in active] or [[]]
    for grp in groups:
        di = self.nc.sync.drain()
        vc = _VectorClock([gc[p] if p in grp else 0 for p in range(_N_PROCS)])
        wait_clock.add_sem_waits(di.ins, _ScopedClock({None: vc}))
    self.nc.all_engine_barrier()
    assert self.sems is not None
    popped = self.nc._tile_sem_poison_stack.pop()
    assert popped is self._sem_poison
    self.nc.clear_and_free_semaphores(list(self.sems.allocated().values()))
    self.nc.all_engine_barrier()


tile.TileContext._drain_and_barrier = _drain_and_barrier_split


_DMA_INST_TYPES = ("InstDMACopy", "InstDMA", "InstDmaTransposeAnt", "InstDMAGatherAnt",
                   "InstTriggeredCopy", "InstCollectiveCompute")


def _split_multi_waits(nc):
    """This walrus build encodes at most ONE sync-wait command per TPB
    instruction (CoreV3GenImpl setupSyncWait fails with "Too many sync wait
    commands" otherwise). Tile attaches several waits to one instruction, so
    hoist all but one wait onto no-op instructions inserted just before, on
    the same engine. For DMAs this gates the descriptor issue on the issuing
    engine instead of the queue, which is more conservative but correct."""
    for f in nc.m.functions:
        for bb in f.blocks:
            insts = bb.instructions
            out = []
            changed = False
            for inst in insts:
                si = inst.sync_info
                if (si is not None and si.on_wait and len(si.on_wait) > 1
                        and inst.engine != mybir.EngineType.Unassigned):
                    waits = list(si.on_wait)
                    for w in waits[:-1]:
                        out.append(mybir.InstNoOp(
                            name=nc.get_next_instruction_name(),
                            engine=inst.engine,
                            sync_info=mybir.SyncInfo(on_wait=[w], on_update=[]),
                            bass_nofuse=True,
                        ))
                    inst.sync_info = mybir.SyncInfo(
                        on_wait=[waits[-1]], on_update=si.on_update)
                    changed = True
                out.append(inst)
            if changed:
                bb.instructions = out


S = 4096
DIM = 1024
INNER = 1024
H = 16
HD = 64
N_CORES = 8
ICPC = INNER // N_CORES  # inner dims per core = 128

F32 = mybir.dt.float32
F32R = mybir.dt.float32r
EXP = mybir.ActivationFunctionType.Exp

LAST_RESULT = None  # BassKernelResults from the most recent run (for test.py)


def _build_nc() -> bass.Bass:
    nc = bass.Bass()

    xT = nc.declare_dram_parameter("xT", [DIM, S], F32R, isOutput=False)
    wq = nc.declare_dram_parameter("wq", [DIM, ICPC], F32R, isOutput=False)
    wk = nc.declare_dram_parameter("wk", [DIM, ICPC], F32R, isOutput=False)
    wv = nc.declare_dram_parameter("wv", [DIM, ICPC], F32R, isOutput=False)
    wo0 = nc.declare_dram_parameter("wo0", [HD, DIM], F32R, isOutput=False)
    wo1 = nc.declare_dram_parameter("wo1", [HD, DIM], F32R, isOutput=False)
    bq = nc.declare_dram_parameter("bq", [ICPC, 1], F32, isOutput=False)
    bk = nc.declare_dram_parameter("bk", [ICPC, 1], F32, isOutput=False)
    ident = nc.declare_dram_parameter("ident", [128, 128], F32R, isOutput=False)
    onesv = nc.declare_dram_parameter("onesv", [128, 128], F32R, isOutput=False)
    out = nc.declare_dram_parameter("out", [S, DIM], F32, isOutput=True)

    def mm(out_ap, lhsT, rhs, **kw):
        nc.tensor.matmul(out_ap, lhsT, rhs, **kw)

    with tile.TileContext(nc) as tc, ExitStack() as ctx:
        xp = ctx.enter_context(tc.tile_pool(name="xp", bufs=2))
        wp = ctx.enter_context(tc.tile_pool(name="wp", bufs=1))
        bigp = ctx.enter_context(tc.tile_pool(name="bigp", bufs=1))
        vtp = ctx.enter_context(tc.tile_pool(name="vtp", bufs=2))
        exp_pool = ctx.enter_context(tc.tile_pool(name="exp_pool", bufs=4))
        rbp = ctx.enter_context(tc.tile_pool(name="rbp", bufs=2))
        denp = ctx.enter_context(tc.tile_pool(name="denp", bufs=2))
        outp = ctx.enter_context(tc.tile_pool(name="outp", bufs=3))
        psA = ctx.enter_context(tc.tile_pool(name="psA", bufs=2, space="PSUM"))
        psU = ctx.enter_context(tc.tile_pool(name="psU", bufs=1, space="PSUM"))
        psM = ctx.enter_context(tc.tile_pool(name="psM", bufs=2, space="PSUM"))

        # --- weights / constants into SBUF ---
        wq_sb = wp.tile([128, 8 * 128], F32R, tag="wq", name="wq_sb")
        wk_sb = wp.tile([128, 8 * 128], F32R, tag="wk", name="wk_sb")
        wv_sb = wp.tile([128, 8 * 128], F32R, tag="wv", name="wv_sb")
        wo0_sb = wp.tile([64, DIM], F32R, tag="wo0", name="wo0_sb")
        wo1_sb = wp.tile([64, DIM], F32R, tag="wo1", name="wo1_sb")
        bq_sb = wp.tile([128, 1], F32, tag="bq", name="bq_sb")
        bk_sb = wp.tile([128, 1], F32, tag="bk", name="bk_sb")
        id_sb = wp.tile([128, 128], F32R, tag="id", name="id_sb")
        ones_sb = wp.tile([128, 128], F32R, tag="ones", name="ones_sb")

        # [DIM, 128] -> [128, dc*128+m] with element (p, dc*128+m) = W[dc*128+p, m]
        nc.sync.dma_start(wq_sb.rearrange("p (c m) -> p c m", c=8),
                          wq.rearrange("(c p) m -> p c m", p=128))
        nc.sync.dma_start(wk_sb.rearrange("p (c m) -> p c m", c=8),
                          wk.rearrange("(c p) m -> p c m", p=128))
        nc.sync.dma_start(wv_sb.rearrange("p (c m) -> p c m", c=8),
                          wv.rearrange("(c p) m -> p c m", p=128))
        nc.sync.dma_start(wo0_sb[:], wo0[:, :])
        nc.sync.dma_start(wo1_sb[:], wo1[:, :])
        nc.sync.dma_start(bq_sb[:], bq[:, :])
        nc.sync.dma_start(bk_sb[:], bk[:, :])
        nc.sync.dma_start(id_sb[:], ident[:, :])
        nc.sync.dma_start(ones_sb[:], onesv[:, :])

        # --- persistent big SBUF tensors ---
        qt_sb = bigp.tile([128, S], F32R, tag="qt", name="qt_sb")
        kt_sb = bigp.tile([128, S], F32R, tag="kt", name="kt_sb")
        ot0_sb = bigp.tile([64, S], F32R, tag="ot0", name="ot0_sb")
        ot1_sb = bigp.tile([64, S], F32R, tag="ot1", name="ot1_sb")
        # V natural tiles: per k-tile 130 cols = [V_h0(64) | ones | V_h1(64) | ones]
        v_sb = bigp.tile([128, 32 * 130], F32R, tag="v", name="v_sb")

        # ones columns of v_sb (cols 64 + 130*t and 129 + 130*t); memset is
        # ISA-invalid for f32r, so copy from the all-ones constant instead
        for t in range(32):
            nc.vector.tensor_copy(v_sb[:, t * 130 + 64 : t * 130 + 65], ones_sb[:, 0:1])
            nc.vector.tensor_copy(v_sb[:, t * 130 + 129 : t * 130 + 130], ones_sb[:, 0:1])


        # ---------------- Phase A: projections ----------------
        # For each 512-wide s-span: load xT[:, span] (all 8 dim-chunks),
        # compute QT/KT/VT [128, 512], transpose VT into V natural tiles.
        for s8 in range(8):
            x8 = xp.tile([128, 8 * 512], F32R, tag="x8", name="x8")
            for dc in range(8):
                nc.sync.dma_start(
                    x8[:, dc * 512 : (dc + 1) * 512],
                    xT[dc * 128 : (dc + 1) * 128, s8 * 512 : (s8 + 1) * 512],
                )
            span = slice(s8 * 512, (s8 + 1) * 512)

            ps_q = psA.tile([128, 512], F32, tag="sc", name="ps_q")
            for dc in range(8):
                mm(ps_q, wq_sb[:, dc * 128 : (dc + 1) * 128],
                   x8[:, dc * 512 : (dc + 1) * 512],
                   start=(dc == 0), stop=(dc == 7))
            nc.vector.tensor_scalar_add(qt_sb[:, span], ps_q, bq_sb[:, 0:1])

            ps_k = psA.tile([128, 512], F32, tag="sc", name="ps_k")
            for dc in range(8):
                mm(ps_k, wk_sb[:, dc * 128 : (dc + 1) * 128],
                   x8[:, dc * 512 : (dc + 1) * 512],
                   start=(dc == 0), stop=(dc == 7))
            nc.vector.tensor_scalar_add(kt_sb[:, span], ps_k, bk_sb[:, 0:1])

            ps_v = psA.tile([128, 512], F32, tag="sc", name="ps_v")
            for dc in range(8):
                mm(ps_v, wv_sb[:, dc * 128 : (dc + 1) * 128],
                   x8[:, dc * 512 : (dc + 1) * 512],
                   start=(dc == 0), stop=(dc == 7))
            vt8 = vtp.tile([128, 512], F32R, tag="vt8", name="vt8")
            nc.vector.tensor_copy(vt8[:], ps_v)

            # transpose VT -> V natural [k, inner] per 128-wide k tile
            for kti in range(4):
                kt = s8 * 4 + kti
                pt = psM.tile([128, 128], F32R, tag="m", name="pt")
                nc.tensor.transpose(pt, vt8[:, kti * 128 : (kti + 1) * 128], id_sb[:])
                base = kt * 130
                nc.vector.tensor_copy(v_sb[:, base : base + 64], pt[:, 0:64])
                nc.vector.tensor_copy(v_sb[:, base + 65 : base + 129], pt[:, 64:128])

        # ---------------- Phase B: attention ----------------
        # The closeout of q-chunk qc is software-pipelined: the PSUM-freeing
        # eviction (DVE only) runs right after qc's kt-loop, but the PE-using
        # closeout (1/den broadcast, normalize, output projection) is emitted
        # AFTER qc+1's kt-loop so those matmuls never stall the PE stream at
        # the qc boundary (which re-throttled HAM to 1.2 GHz for ~50us each).
        def closeout2(qc, rdens):
            qspan = slice(qc * 512, (qc + 1) * 512)
            for h, ot_h in ((0, ot0_sb), (1, ot1_sb)):
                rb_ps = psM.tile([128, 512], F32, tag="m", name="rb_ps")
                mm(rb_ps, ones_sb[64:65, :], rdens[h][64:65, :],
                   start=True, stop=True)
                rb = rbp.tile([128, 512], F32, tag="rb", name="rb")
                nc.vector.tensor_copy(rb[:], rb_ps)
                nc.vector.tensor_mul(ot_h[:, qspan], ot_h[:, qspan],
                                     rb[0:64, :])
            for sti in range(4):
                st = qc * 4 + sti
                for half in (0, 1):
                    z = psM.tile([128, 512], F32, tag="m", name="z")
                    mm(z, ot0_sb[:, st * 128 : (st + 1) * 128],
                       wo0_sb[:, half * 512 : (half + 1) * 512],
                       start=True, stop=False)
                    mm(z, ot1_sb[:, st * 128 : (st + 1) * 128],
                       wo1_sb[:, half * 512 : (half + 1) * 512],
                       start=False, stop=True)
                    ob = outp.tile([128, 512], F32, tag="ob", name="ob")
                    nc.vector.tensor_copy(ob[:], z)
                    nc.sync.dma_start(
                        out[st * 128 : (st + 1) * 128,
                            half * 512 : (half + 1) * 512],
                        ob[:],
                    )

        def emit_recips(dens):
            # deferred: the 3.3us single-lane reciprocals; their result is
            # only needed by the NEXT boundary's closeout, so they go at the
            # tail of the boundary's DVE work.
            rdens = {}
            for h in (0, 1):
                den2 = denp.tile([65, 512], F32, tag="den2", name="den2")
                nc.vector.reciprocal(den2[64:65, :], dens[h][64:65, :])
                rden = denp.tile([65, 512], F32R, tag="rden", name="rden", bufs=4)
                nc.vector.tensor_copy(rden[64:65, :], den2[64:65, :])
                rdens[h] = rden
            return rdens

        prev_closeout = None
        for qc in range(8):
            qspan = slice(qc * 512, (qc + 1) * 512)
            u0 = psU.tile([65, 512], F32, tag="u0", name="u0")
            u1 = psU.tile([65, 512], F32, tag="u1", name="u1")
            for kt in range(32):
                ps = psA.tile([128, 1024], F32, tag="sc", name="ps_s")
                # scores^T [k, q] for both heads (row groups 0:64 / 64:128)
                mm(ps[:, 0:512],
                   kt_sb[0:64, kt * 128 : (kt + 1) * 128],
                   qt_sb[0:64, qspan], start=True, stop=True)
                mm(ps[:, 512:1024],
                   kt_sb[64:128, kt * 128 : (kt + 1) * 128],
                   qt_sb[64:128, qspan], start=True, stop=True)
                ex = exp_pool.tile([128, 1024], F32R, tag="ex", name="ex")
                nc.scalar.activation(ex[:], ps[:], EXP)
                base = kt * 130
                mm(u0, v_sb[:, base : base + 65], ex[:, 0:512],
                   start=(kt == 0), stop=(kt == 31))
                mm(u1, v_sb[:, base + 65 : base + 130], ex[:, 512:1024],
                   start=(kt == 0), stop=(kt == 31))

            # Boundary DVE order matters (DVE executes in emission order):
            # 1) urgent: evict U PSUM (unblocks next chunk's attn@V)
            # 2) previous chunk's closeout (unblocks its out-proj matmuls,
            #    which are what keeps the PE fed across this boundary)
            # 3) deferred: this chunk's reciprocals (needed one boundary later)
            dens = {}
            for h, u_h, ot_h in ((0, u0, ot0_sb), (1, u1, ot1_sb)):
                nc.vector.tensor_copy(ot_h[:, qspan], u_h[0:64, :])
                den = denp.tile([65, 512], F32, tag="den", name="den", bufs=4)
                nc.vector.tensor_copy(den[64:65, :], u_h[64:65, :])
                dens[h] = den

            if prev_closeout is not None:
                prev_closeout()
            rdens = emit_recips(dens)
            prev_closeout = (lambda q, r: lambda: closeout2(q, r))(qc, rdens)
        prev_closeout()

    _split_multi_waits(nc)
    return nc


_NC_CACHE: dict = {}


def kernel(x, Wq, bq, Wk, bk, Wv, bv, Wo, bo):
    global LAST_RESULT
    x = np.asarray(x, dtype=np.float32)
    Wq = np.asarray(Wq, dtype=np.float32)
    Wk = np.asarray(Wk, dtype=np.float32)
    Wv = np.asarray(Wv, dtype=np.float32)
    Wo = np.asarray(Wo, dtype=np.float32)
    bq = np.asarray(bq, dtype=np.float32)
    bk = np.asarray(bk, dtype=np.float32)
    bv = np.asarray(bv, dtype=np.float32)
    bo = np.asarray(bo, dtype=np.float32)

    if "nc" not in _NC_CACHE:
        _NC_CACHE["nc"] = _build_nc()
    nc = _NC_CACHE["nc"]

    xT = np.ascontiguousarray(x.reshape(S, DIM).T)
    ident = np.eye(128, dtype=np.float32)
    onesv = np.ones((128, 128), dtype=np.float32)

    in_maps = []
    for c in range(N_CORES):
        sl = slice(c * ICPC, (c + 1) * ICPC)
        in_maps.append({
            "xT": xT,
            "wq": np.ascontiguousarray(Wq[:, sl]),
            "wk": np.ascontiguousarray(Wk[:, sl]),
            "wv": np.ascontiguousarray(Wv[:, sl]),
            "wo0": np.ascontiguousarray(Wo[c * ICPC : c * ICPC + HD, :]),
            "wo1": np.ascontiguousarray(Wo[c * ICPC + HD : (c + 1) * ICPC, :]),
            "bq": np.ascontiguousarray(bq[sl].reshape(ICPC, 1)),
            "bk": np.ascontiguousarray(bk[sl].reshape(ICPC, 1)),
            "ident": ident,
            "onesv": onesv,
        })

    res = run_bass_kernel_spmd(
        nc, in_maps, core_ids=list(range(N_CORES)),
        trace=bool(int(os.environ.get("KERNEL_TRACE", "0"))),
    )
    LAST_RESULT = res

    acc = np.zeros((S, DIM), dtype=np.float64)
    for r in res.results:
        acc += r["out"].astype(np.float64)
    # exact linear corrections handled on host: v-bias and output bias
    acc += (bv.astype(np.float64) @ Wo.astype(np.float64))[None, :]
    acc += bo.astype(np.float64)[None, :]
    return acc.astype(np.float32).reshape(1, S, DIM)


# revision 22
# speedup vs baseline: 1.3879x; 1.1569x over previous
"""Trainium2 Bass kernel for nn_Attention_5042291605734.

Full multi-head attention (B=1, S=4096, DIM=1024, H=16, HD=64), fp32.

Sharding: 2 heads per core (tensor-parallel on inner dim). Each core
computes q/k/v projections for its 128-wide inner slice, full softmax
attention for its 2 heads, and a partial output projection against its
128-row slice of Wo. The host sums the 8 partial outputs and adds the
bias terms (bv @ Wo + bo), which are exactly linear corrections.

Per-core dataflow (all layouts chosen so no on-chip transposes of big
tensors are needed; host passes x pre-transposed):
  xT [DIM, S] (host-transposed)  --mm-->  QT/KT/VT [128, S] (inner on
  partitions).  VT is PE-transposed per 128-tile into V natural [k, hd]
  with an appended ones column (so attn@V also produces the softmax
  denominator as row 64 of the PSUM accumulator).
  Scores are computed in [k, q] layout: ST_tile = KT_slice.T @ QT.
  exp() runs on the Scalar engine PSUM->SBUF; no max-subtraction is
  needed (|scores| <= ~16, exp stays well inside fp32 range).
  U^T[hd(+den), q] += V_aug.T @ exp(ST) accumulated over k tiles.
  Normalization multiplies U^T rows by a PE-broadcast of 1/den.
  Output projection: out[s, :] = O^T[:, s_tile].T @ Wo_c.
"""

import os
from contextlib import ExitStack

import numpy as np

import concourse.bass as bass
import concourse.mybir as mybir
import concourse.tile as tile
from concourse.bass_utils import run_bass_kernel_spmd
from concourse.vector_clock import ScopedClock as _ScopedClock, VectorClock as _VectorClock

_N_PROCS = 27


def _drain_and_barrier_split(self, tick_clock, wait_clock):
    """Replacement for TileContext._drain_and_barrier: the stock version puts
    every live proc's final tick on ONE drain instruction, and this walrus
    build rejects instructions with >2 sync-wait commands (CoreV3GenImpl
    setupSyncWait "Too many sync wait commands"). Split the global-clock wait
    across several sequential drains, one proc each."""
    gc = tick_clock.global_clock
    active = [p for p in range(_N_PROCS) if gc[p] > 0]
    groups = [[p] for p in active] or [[]]
    for grp in groups:
        di = self.nc.sync.drain()
        vc = _VectorClock([gc[p] if p in grp else 0 for p in range(_N_PROCS)])
        wait_clock.add_sem_waits(di.ins, _ScopedClock({None: vc}))
    self.nc.all_engine_barrier()
    assert self.sems is not None
    popped = self.nc._tile_sem_poison_stack.pop()
    assert popped is self._sem_poison
    self.nc.clear_and_free_semaphores(list(self.sems.allocated().values()))
    self.nc.all_engine_barrier()


tile.TileContext._drain_and_barrier = _drain_and_barrier_split


_DMA_INST_TYPES = ("InstDMACopy", "InstDMA", "InstDmaTransposeAnt", "InstDMAGatherAnt",
                   "InstTriggeredCopy", "InstCollectiveCompute")


def _split_multi_waits(nc):
    """This walrus build encodes at most ONE sync-wait command per TPB
    instruction (CoreV3GenImpl setupSyncWait fails with "Too many sync wait
    commands" otherwise). Tile attaches several waits to one instruction, so
    hoist all but one wait onto no-op instructions inserted just before, on
    the same engine. For DMAs this gates the descriptor issue on the issuing
    engine instead of the queue, which is more conservative but correct."""
    for f in nc.m.functions:
        for bb in f.blocks:
            insts = bb.instructions
            out = []
            changed = False
            for inst in insts:
                si = inst.sync_info
                if (si is not None and si.on_wait and len(si.on_wait) > 1
                        and inst.engine != mybir.EngineType.Unassigned):
                    waits = list(si.on_wait)
                    for w in waits[:-1]:
                        out.append(mybir.InstNoOp(
                            name=nc.get_next_instruction_name(),
                            engine=inst.engine,
                            sync_info=mybir.SyncInfo(on_wait=[w], on_update=[]),
                            bass_nofuse=True,
                        ))
                    inst.sync_info = mybir.SyncInfo(
                        on_wait=[waits[-1]], on_update=si.on_update)
                    changed = True
                out.append(inst)
            if changed:
                bb.instructions = out


S = 4096
DIM = 1024
INNER = 1024
H = 16
HD = 64
N_CORES = 8
ICPC = INNER // N_CORES  # inner dims per core = 128

F32 = mybir.dt.float32
F32R = mybir.dt.float32r
BF16 = mybir.dt.bfloat16
EXP = mybir.ActivationFunctionType.Exp

LAST_RESULT = None  # BassKernelResults from the most recent run (for test.py)


def _build_nc() -> bass.Bass:
    nc = bass.Bass()

    xT = nc.declare_dram_parameter("xT", [DIM, S], F32R, isOutput=False)
    wq = nc.declare_dram_parameter("wq", [DIM, ICPC], F32R, isOutput=False)
    wk = nc.declare_dram_parameter("wk", [DIM, ICPC], F32R, isOutput=False)
    wv = nc.declare_dram_parameter("wv", [DIM, ICPC], F32R, isOutput=False)
    wo0 = nc.declare_dram_parameter("wo0", [HD, DIM], F32R, isOutput=False)
    wo1 = nc.declare_dram_parameter("wo1", [HD, DIM], F32R, isOutput=False)
    bq = nc.declare_dram_parameter("bq", [ICPC, 1], F32, isOutput=False)
    bk = nc.declare_dram_parameter("bk", [ICPC, 1], F32, isOutput=False)
    ident = nc.declare_dram_parameter("ident", [128, 128], F32R, isOutput=False)
    onesv = nc.declare_dram_parameter("onesv", [128, 128], F32R, isOutput=False)
    out = nc.declare_dram_parameter("out", [S, DIM], F32, isOutput=True)

    def mm(out_ap, lhsT, rhs, **kw):
        nc.tensor.matmul(out_ap, lhsT, rhs, **kw)

    with tile.TileContext(nc) as tc, ExitStack() as ctx:
        xp = ctx.enter_context(tc.tile_pool(name="xp", bufs=2))
        wp = ctx.enter_context(tc.tile_pool(name="wp", bufs=1))
        bigp = ctx.enter_context(tc.tile_pool(name="bigp", bufs=1))
        vtp = ctx.enter_context(tc.tile_pool(name="vtp", bufs=2))
        exp_pool = ctx.enter_context(tc.tile_pool(name="exp_pool", bufs=4))
        rbp = ctx.enter_context(tc.tile_pool(name="rbp", bufs=2))
        denp = ctx.enter_context(tc.tile_pool(name="denp", bufs=2))
        outp = ctx.enter_context(tc.tile_pool(name="outp", bufs=3))
        psA = ctx.enter_context(tc.tile_pool(name="psA", bufs=2, space="PSUM"))
        psU = ctx.enter_context(tc.tile_pool(name="psU", bufs=1, space="PSUM"))
        psM = ctx.enter_context(tc.tile_pool(name="psM", bufs=2, space="PSUM"))

        # --- weights / constants into SBUF ---
        wq_sb = wp.tile([128, 8 * 128], F32R, tag="wq", name="wq_sb")
        wk_sb = wp.tile([128, 8 * 128], F32R, tag="wk", name="wk_sb")
        wv_sb = wp.tile([128, 8 * 128], F32R, tag="wv", name="wv_sb")
        wo0_sb = wp.tile([64, DIM], F32R, tag="wo0", name="wo0_sb")
        wo1_sb = wp.tile([64, DIM], F32R, tag="wo1", name="wo1_sb")
        bq_sb = wp.tile([128, 1], F32, tag="bq", name="bq_sb")
        bk_sb = wp.tile([128, 1], F32, tag="bk", name="bk_sb")
        id_sb = wp.tile([128, 128], F32R, tag="id", name="id_sb")
        ones_sb = wp.tile([128, 128], F32R, tag="ones", name="ones_sb")

        # [DIM, 128] -> [128, dc*128+m] with element (p, dc*128+m) = W[dc*128+p, m]
        nc.sync.dma_start(wq_sb.rearrange("p (c m) -> p c m", c=8),
                          wq.rearrange("(c p) m -> p c m", p=128))
        nc.sync.dma_start(wk_sb.rearrange("p (c m) -> p c m", c=8),
                          wk.rearrange("(c p) m -> p c m", p=128))
        nc.sync.dma_start(wv_sb.rearrange("p (c m) -> p c m", c=8),
                          wv.rearrange("(c p) m -> p c m", p=128))
        nc.sync.dma_start(wo0_sb[:], wo0[:, :])
        nc.sync.dma_start(wo1_sb[:], wo1[:, :])
        nc.sync.dma_start(bq_sb[:], bq[:, :])
        nc.sync.dma_start(bk_sb[:], bk[:, :])
        nc.sync.dma_start(id_sb[:], ident[:, :])
        nc.sync.dma_start(ones_sb[:], onesv[:, :])

        # --- persistent big SBUF tensors ---
        qt_sb = bigp.tile([128, S], F32R, tag="qt", name="qt_sb")
        kt_sb = bigp.tile([128, S], F32R, tag="kt", name="kt_sb")
        ot0_sb = bigp.tile([64, S], F32R, tag="ot0", name="ot0_sb")
        ot1_sb = bigp.tile([64, S], F32R, tag="ot1", name="ot1_sb")
        # V natural tiles: per k-tile 130 cols = [V_h0(64) | ones | V_h1(64) | ones]
        v_sb = bigp.tile([128, 32 * 130], BF16, tag="v", name="v_sb")

        # ones columns of v_sb (cols 64 + 130*t and 129 + 130*t); memset is
        # ISA-invalid for f32r, so copy from the all-ones constant instead
        for t in range(32):
            nc.vector.tensor_copy(v_sb[:, t * 130 + 64 : t * 130 + 65], ones_sb[:, 0:1])
            nc.vector.tensor_copy(v_sb[:, t * 130 + 129 : t * 130 + 130], ones_sb[:, 0:1])


        # ---------------- Phase A: projections ----------------
        # For each 512-wide s-span: load xT[:, span] (all 8 dim-chunks),
        # compute QT/KT/VT [128, 512], transpose VT into V natural tiles.
        for s8 in range(8):
            x8 = xp.tile([128, 8 * 512], F32R, tag="x8", name="x8")
            for dc in range(8):
                nc.sync.dma_start(
                    x8[:, dc * 512 : (dc + 1) * 512],
                    xT[dc * 128 : (dc + 1) * 128, s8 * 512 : (s8 + 1) * 512],
                )
            span = slice(s8 * 512, (s8 + 1) * 512)

            def proj(ps_t, w_sb):
                for dc in range(8):
                    mm(ps_t,
                       w_sb[:, dc * 128 : (dc + 1) * 128],
                       x8[:, dc * 512 : (dc + 1) * 512],
                       start=(dc == 0), stop=(dc == 7))

            ps_q = psA.tile([128, 512], F32, tag="sc", name="ps_q")
            proj(ps_q, wq_sb)
            nc.vector.tensor_scalar_add(qt_sb[:, span], ps_q, bq_sb[:, 0:1])

            ps_k = psA.tile([128, 512], F32, tag="sc", name="ps_k")
            proj(ps_k, wk_sb)
            nc.vector.tensor_scalar_add(kt_sb[:, span], ps_k, bk_sb[:, 0:1])

            ps_v = psA.tile([128, 512], F32, tag="sc", name="ps_v")
            proj(ps_v, wv_sb)
            vt8 = vtp.tile([128, 512], F32R, tag="vt8", name="vt8")
            nc.vector.tensor_copy(vt8[:], ps_v)

            # transpose VT -> V natural [k, inner] per 128-wide k tile
            for kti in range(4):
                kt = s8 * 4 + kti
                pt = psM.tile([128, 128], F32R, tag="m", name="pt")
                nc.tensor.transpose(pt, vt8[:, kti * 128 : (kti + 1) * 128], id_sb[:])
                base = kt * 130
                nc.vector.tensor_copy(v_sb[:, base : base + 64], pt[:, 0:64])
                nc.vector.tensor_copy(v_sb[:, base + 65 : base + 129], pt[:, 64:128])

        # ---------------- Phase B: attention ----------------
        # The closeout of q-chunk qc is software-pipelined: the PSUM-freeing
        # eviction (DVE only) runs right after qc's kt-loop, but the PE-using
        # closeout (1/den broadcast, normalize, output projection) is emitted
        # AFTER qc+1's kt-loop so those matmuls never stall the PE stream at
        # the qc boundary (which re-throttled HAM to 1.2 GHz for ~50us each).
        def closeout2(qc, rdens):
            qspan = slice(qc * 512, (qc + 1) * 512)
            for h, ot_h in ((0, ot0_sb), (1, ot1_sb)):
                rb_ps = psM.tile([128, 512], F32, tag="m", name="rb_ps")
                mm(rb_ps, ones_sb[64:65, :], rdens[h][64:65, :],
                   start=True, stop=True)
                rb = rbp.tile([128, 512], F32, tag="rb", name="rb")
                nc.vector.tensor_copy(rb[:], rb_ps)
                nc.vector.tensor_mul(ot_h[:, qspan], ot_h[:, qspan],
                                     rb[0:64, :])
            for sti in range(4):
                st = qc * 4 + sti
                for half in (0, 1):
                    z = psM.tile([128, 512], F32, tag="m", name="z")
                    mm(z, ot0_sb[:, st * 128 : (st + 1) * 128],
                       wo0_sb[:, half * 512 : (half + 1) * 512],
                       start=True, stop=False)
                    mm(z, ot1_sb[:, st * 128 : (st + 1) * 128],
                       wo1_sb[:, half * 512 : (half + 1) * 512],
                       start=False, stop=True)
                    ob = outp.tile([128, 512], F32, tag="ob", name="ob")
                    nc.vector.tensor_copy(ob[:], z)
                    nc.sync.dma_start(
                        out[st * 128 : (st + 1) * 128,
                            half * 512 : (half + 1) * 512],
                        ob[:],
                    )

        def emit_recips(dens):
            # deferred: the 3.3us single-lane reciprocals; their result is
            # only needed by the NEXT boundary's closeout, so they go at the
            # tail of the boundary's DVE work.
            rdens = {}
            for h in (0, 1):
                den2 = denp.tile([65, 512], F32, tag="den2", name="den2")
                nc.vector.reciprocal(den2[64:65, :], dens[h][64:65, :])
                rden = denp.tile([65, 512], F32R, tag="rden", name="rden", bufs=4)
                nc.vector.tensor_copy(rden[64:65, :], den2[64:65, :])
                rdens[h] = rden
            return rdens

        prev_closeout = None
        for qc in range(8):
            qspan = slice(qc * 512, (qc + 1) * 512)
            u0 = psU.tile([65, 512], F32, tag="u0", name="u0")
            u1 = psU.tile([65, 512], F32, tag="u1", name="u1")
            for kt in range(32):
                ps = psA.tile([128, 1024], F32, tag="sc", name="ps_s")
                # scores^T [k, q] for both heads (row groups 0:64 / 64:128)
                mm(ps[:, 0:512],
                   kt_sb[0:64, kt * 128 : (kt + 1) * 128],
                   qt_sb[0:64, qspan], start=True, stop=True)
                mm(ps[:, 512:1024],
                   kt_sb[64:128, kt * 128 : (kt + 1) * 128],
                   qt_sb[64:128, qspan], start=True, stop=True)
                ex = exp_pool.tile([128, 1024], BF16, tag="ex", name="ex")
                nc.scalar.activation(ex[:], ps[:], EXP)
                base = kt * 130
                mm(u0, v_sb[:, base : base + 65], ex[:, 0:512],
                   start=(kt == 0), stop=(kt == 31))
                mm(u1, v_sb[:, base + 65 : base + 130], ex[:, 512:1024],
                   start=(kt == 0), stop=(kt == 31))

            # Boundary DVE order matters (DVE executes in emission order):
            # 1) urgent: evict U PSUM (unblocks next chunk's attn@V)
            # 2) previous chunk's closeout (unblocks its out-proj matmuls,
            #    which are what keeps the PE fed across this boundary)
            # 3) deferred: this chunk's reciprocals (needed one boundary later)
            dens = {}
            for h, u_h, ot_h in ((0, u0, ot0_sb), (1, u1, ot1_sb)):
                nc.vector.tensor_copy(ot_h[:, qspan], u_h[0:64, :])
                den = denp.tile([65, 512], F32, tag="den", name="den", bufs=4)
                nc.vector.tensor_copy(den[64:65, :], u_h[64:65, :])
                dens[h] = den

            if prev_closeout is not None:
                prev_closeout()
            rdens = emit_recips(dens)
            prev_closeout = (lambda q, r: lambda: closeout2(q, r))(qc, rdens)
        prev_closeout()

    _split_multi_waits(nc)
    return nc


_NC_CACHE: dict = {}


def kernel(x, Wq, bq, Wk, bk, Wv, bv, Wo, bo):
    global LAST_RESULT
    x = np.asarray(x, dtype=np.float32)
    Wq = np.asarray(Wq, dtype=np.float32)
    Wk = np.asarray(Wk, dtype=np.float32)
    Wv = np.asarray(Wv, dtype=np.float32)
    Wo = np.asarray(Wo, dtype=np.float32)
    bq = np.asarray(bq, dtype=np.float32)
    bk = np.asarray(bk, dtype=np.float32)
    bv = np.asarray(bv, dtype=np.float32)
    bo = np.asarray(bo, dtype=np.float32)

    if "nc" not in _NC_CACHE:
        _NC_CACHE["nc"] = _build_nc()
    nc = _NC_CACHE["nc"]

    xT = np.ascontiguousarray(x.reshape(S, DIM).T)
    ident = np.eye(128, dtype=np.float32)
    onesv = np.ones((128, 128), dtype=np.float32)

    in_maps = []
    for c in range(N_CORES):
        sl = slice(c * ICPC, (c + 1) * ICPC)
        in_maps.append({
            "xT": xT,
            "wq": np.ascontiguousarray(Wq[:, sl]),
            "wk": np.ascontiguousarray(Wk[:, sl]),
            "wv": np.ascontiguousarray(Wv[:, sl]),
            "wo0": np.ascontiguousarray(Wo[c * ICPC : c * ICPC + HD, :]),
            "wo1": np.ascontiguousarray(Wo[c * ICPC + HD : (c + 1) * ICPC, :]),
            "bq": np.ascontiguousarray(bq[sl].reshape(ICPC, 1)),
            "bk": np.ascontiguousarray(bk[sl].reshape(ICPC, 1)),
            "ident": ident,
            "onesv": onesv,
        })

    res = run_bass_kernel_spmd(
        nc, in_maps, core_ids=list(range(N_CORES)),
        trace=bool(int(os.environ.get("KERNEL_TRACE", "0"))),
    )
    LAST_RESULT = res

    acc = np.zeros((S, DIM), dtype=np.float64)
    for r in res.results:
        acc += r["out"].astype(np.float64)
    # exact linear corrections handled on host: v-bias and output bias
    acc += (bv.astype(np.float64) @ Wo.astype(np.float64))[None, :]
    acc += bo.astype(np.float64)[None, :]
    return acc.astype(np.float32).reshape(1, S, DIM)


# revision 23
# speedup vs baseline: 1.5383x; 1.1084x over previous
"""Trainium2 Bass kernel for nn_Attention_5042291605734.

Full multi-head attention (B=1, S=4096, DIM=1024, H=16, HD=64), fp32.

Sharding: 2 heads per core (tensor-parallel on inner dim). Each core
computes q/k/v projections for its 128-wide inner slice, full softmax
attention for its 2 heads, and a partial output projection against its
128-row slice of Wo. The host sums the 8 partial outputs and adds the
bias terms (bv @ Wo + bo), which are exactly linear corrections.

Per-core dataflow (all layouts chosen so no on-chip transposes of big
tensors are needed; host passes x pre-transposed):
  xT [DIM, S] (host-transposed)  --mm-->  QT/KT/VT [128, S] (inner on
  partitions).  VT is PE-transposed per 128-tile into V natural [k, hd]
  with an appended ones column (so attn@V also produces the softmax
  denominator as row 64 of the PSUM accumulator).
  Scores are computed in [k, q] layout: ST_tile = KT_slice.T @ QT.
  exp() runs on the Scalar engine PSUM->SBUF; no max-subtraction is
  needed (|scores| <= ~16, exp stays well inside fp32 range).
  U^T[hd(+den), q] += V_aug.T @ exp(ST) accumulated over k tiles.
  Normalization multiplies U^T rows by a PE-broadcast of 1/den.
  Output projection: out[s, :] = O^T[:, s_tile].T @ Wo_c.
"""

import os
from contextlib import ExitStack

import numpy as np

import concourse.bass as bass
import concourse.mybir as mybir
import concourse.tile as tile
from concourse.bass_utils import run_bass_kernel_spmd
from concourse.vector_clock import ScopedClock as _ScopedClock, VectorClock as _VectorClock

_N_PROCS = 27


def _drain_and_barrier_split(self, tick_clock, wait_clock):
    """Replacement for TileContext._drain_and_barrier: the stock version puts
    every live proc's final tick on ONE drain instruction, and this walrus
    build rejects instructions with >2 sync-wait commands (CoreV3GenImpl
    setupSyncWait "Too many sync wait commands"). Split the global-clock wait
    across several sequential drains, one proc each."""
    gc = tick_clock.global_clock
    active = [p for p in range(_N_PROCS) if gc[p] > 0]
    groups = [[p] for p in active] or [[]]
    for grp in groups:
        di = self.nc.sync.drain()
        vc = _VectorClock([gc[p] if p in grp else 0 for p in range(_N_PROCS)])
        wait_clock.add_sem_waits(di.ins, _ScopedClock({None: vc}))
    self.nc.all_engine_barrier()
    assert self.sems is not None
    popped = self.nc._tile_sem_poison_stack.pop()
    assert popped is self._sem_poison
    self.nc.clear_and_free_semaphores(list(self.sems.allocated().values()))
    self.nc.all_engine_barrier()


tile.TileContext._drain_and_barrier = _drain_and_barrier_split


_DMA_INST_TYPES = ("InstDMACopy", "InstDMA", "InstDmaTransposeAnt", "InstDMAGatherAnt",
                   "InstTriggeredCopy", "InstCollectiveCompute")


def _split_multi_waits(nc):
    """This walrus build encodes at most ONE sync-wait command per TPB
    instruction (CoreV3GenImpl setupSyncWait fails with "Too many sync wait
    commands" otherwise). Tile attaches several waits to one instruction, so
    hoist all but one wait onto no-op instructions inserted just before, on
    the same engine. For DMAs this gates the descriptor issue on the issuing
    engine instead of the queue, which is more conservative but correct."""
    for f in nc.m.functions:
        for bb in f.blocks:
            insts = bb.instructions
            out = []
            changed = False
            for inst in insts:
                si = inst.sync_info
                if (si is not None and si.on_wait and len(si.on_wait) > 1
                        and inst.engine != mybir.EngineType.Unassigned):
                    waits = list(si.on_wait)
                    for w in waits[:-1]:
                        out.append(mybir.InstNoOp(
                            name=nc.get_next_instruction_name(),
                            engine=inst.engine,
                            sync_info=mybir.SyncInfo(on_wait=[w], on_update=[]),
                            bass_nofuse=True,
                        ))
                    inst.sync_info = mybir.SyncInfo(
                        on_wait=[waits[-1]], on_update=si.on_update)
                    changed = True
                out.append(inst)
            if changed:
                bb.instructions = out


S = 4096
DIM = 1024
INNER = 1024
H = 16
HD = 64
N_CORES = 8
ICPC = INNER // N_CORES  # inner dims per core = 128

F32 = mybir.dt.float32
F32R = mybir.dt.float32r
BF16 = mybir.dt.bfloat16
EXP = mybir.ActivationFunctionType.Exp

LAST_RESULT = None  # BassKernelResults from the most recent run (for test.py)


def _build_nc() -> bass.Bass:
    nc = bass.Bass()

    xT = nc.declare_dram_parameter("xT", [DIM, S], F32R, isOutput=False)
    wq = nc.declare_dram_parameter("wq", [DIM, ICPC], F32R, isOutput=False)
    wk = nc.declare_dram_parameter("wk", [DIM, ICPC], F32R, isOutput=False)
    wv = nc.declare_dram_parameter("wv", [DIM, ICPC], F32R, isOutput=False)
    wo0 = nc.declare_dram_parameter("wo0", [HD, DIM], F32R, isOutput=False)
    wo1 = nc.declare_dram_parameter("wo1", [HD, DIM], F32R, isOutput=False)
    bq = nc.declare_dram_parameter("bq", [ICPC, 1], F32, isOutput=False)
    bk = nc.declare_dram_parameter("bk", [ICPC, 1], F32, isOutput=False)
    ident = nc.declare_dram_parameter("ident", [128, 128], F32R, isOutput=False)
    onesv = nc.declare_dram_parameter("onesv", [128, 128], F32R, isOutput=False)
    out = nc.declare_dram_parameter("out", [S, DIM], F32, isOutput=True)

    def mm(out_ap, lhsT, rhs, **kw):
        nc.tensor.matmul(out_ap, lhsT, rhs, **kw)

    with tile.TileContext(nc) as tc, ExitStack() as ctx:
        xp = ctx.enter_context(tc.tile_pool(name="xp", bufs=2))
        wp = ctx.enter_context(tc.tile_pool(name="wp", bufs=1))
        bigp = ctx.enter_context(tc.tile_pool(name="bigp", bufs=1))
        vtp = ctx.enter_context(tc.tile_pool(name="vtp", bufs=2))
        exp_pool = ctx.enter_context(tc.tile_pool(name="exp_pool", bufs=4))
        rbp = ctx.enter_context(tc.tile_pool(name="rbp", bufs=2))
        denp = ctx.enter_context(tc.tile_pool(name="denp", bufs=2))
        outp = ctx.enter_context(tc.tile_pool(name="outp", bufs=3))
        psA = ctx.enter_context(tc.tile_pool(name="psA", bufs=2, space="PSUM"))
        psU = ctx.enter_context(tc.tile_pool(name="psU", bufs=1, space="PSUM"))
        psM = ctx.enter_context(tc.tile_pool(name="psM", bufs=2, space="PSUM"))

        # --- weights / constants into SBUF ---
        wq_sb = wp.tile([128, 8 * 128], F32R, tag="wq", name="wq_sb")
        wk_sb = wp.tile([128, 8 * 128], F32R, tag="wk", name="wk_sb")
        wv_sb = wp.tile([128, 8 * 128], F32R, tag="wv", name="wv_sb")
        wo0_sb = wp.tile([64, DIM], F32R, tag="wo0", name="wo0_sb")
        wo1_sb = wp.tile([64, DIM], F32R, tag="wo1", name="wo1_sb")
        bq_sb = wp.tile([128, 1], F32, tag="bq", name="bq_sb")
        bk_sb = wp.tile([128, 1], F32, tag="bk", name="bk_sb")
        id_sb = wp.tile([128, 128], F32R, tag="id", name="id_sb")
        ones_sb = wp.tile([128, 128], F32R, tag="ones", name="ones_sb")

        # [DIM, 128] -> [128, dc*128+m] with element (p, dc*128+m) = W[dc*128+p, m]
        nc.sync.dma_start(wq_sb.rearrange("p (c m) -> p c m", c=8),
                          wq.rearrange("(c p) m -> p c m", p=128))
        nc.sync.dma_start(wk_sb.rearrange("p (c m) -> p c m", c=8),
                          wk.rearrange("(c p) m -> p c m", p=128))
        nc.sync.dma_start(wv_sb.rearrange("p (c m) -> p c m", c=8),
                          wv.rearrange("(c p) m -> p c m", p=128))
        nc.sync.dma_start(wo0_sb[:], wo0[:, :])
        nc.sync.dma_start(wo1_sb[:], wo1[:, :])
        nc.sync.dma_start(bq_sb[:], bq[:, :])
        nc.sync.dma_start(bk_sb[:], bk[:, :])
        nc.sync.dma_start(id_sb[:], ident[:, :])
        nc.sync.dma_start(ones_sb[:], onesv[:, :])

        # --- persistent big SBUF tensors ---
        qt_sb = bigp.tile([128, S], F32R, tag="qt", name="qt_sb")
        kt_sb = bigp.tile([128, S], F32R, tag="kt", name="kt_sb")
        ot0_sb = bigp.tile([64, S], F32R, tag="ot0", name="ot0_sb")
        ot1_sb = bigp.tile([64, S], F32R, tag="ot1", name="ot1_sb")
        # V natural tiles: per k-tile 130 cols = [V_h0(64) | ones | V_h1(64) | ones]
        v_sb = bigp.tile([128, 32 * 130], BF16, tag="v", name="v_sb")

        # ones columns of v_sb (cols 64 + 130*t and 129 + 130*t); memset is
        # ISA-invalid for f32r, so copy from the all-ones constant instead
        for t in range(32):
            nc.vector.tensor_copy(v_sb[:, t * 130 + 64 : t * 130 + 65], ones_sb[:, 0:1])
            nc.vector.tensor_copy(v_sb[:, t * 130 + 129 : t * 130 + 130], ones_sb[:, 0:1])


        # ---------------- Phase A: projections ----------------
        # For each 512-wide s-span: load xT[:, span] (all 8 dim-chunks),
        # compute QT/KT/VT [128, 512], transpose VT into V natural tiles.
        for s8 in range(8):
            x8 = xp.tile([128, 8 * 512], F32R, tag="x8", name="x8")
            for dc in range(8):
                nc.sync.dma_start(
                    x8[:, dc * 512 : (dc + 1) * 512],
                    xT[dc * 128 : (dc + 1) * 128, s8 * 512 : (s8 + 1) * 512],
                )
            span = slice(s8 * 512, (s8 + 1) * 512)

            def proj(ps_t, w_sb):
                for dc in range(8):
                    mm(ps_t,
                       w_sb[:, dc * 128 : (dc + 1) * 128],
                       x8[:, dc * 512 : (dc + 1) * 512],
                       start=(dc == 0), stop=(dc == 7))

            ps_q = psA.tile([128, 512], F32, tag="sc", name="ps_q")
            proj(ps_q, wq_sb)
            nc.vector.tensor_scalar_add(qt_sb[:, span], ps_q, bq_sb[:, 0:1])

            ps_k = psA.tile([128, 512], F32, tag="sc", name="ps_k")
            proj(ps_k, wk_sb)
            nc.vector.tensor_scalar_add(kt_sb[:, span], ps_k, bk_sb[:, 0:1])

            ps_v = psA.tile([128, 512], F32, tag="sc", name="ps_v")
            proj(ps_v, wv_sb)
            vt8 = vtp.tile([128, 512], F32R, tag="vt8", name="vt8")
            nc.vector.tensor_copy(vt8[:], ps_v)

            # transpose VT -> V natural [k, inner] per 128-wide k tile
            for kti in range(4):
                kt = s8 * 4 + kti
                pt = psM.tile([128, 128], F32R, tag="m", name="pt")
                nc.tensor.transpose(pt, vt8[:, kti * 128 : (kti + 1) * 128], id_sb[:])
                base = kt * 130
                nc.vector.tensor_copy(v_sb[:, base : base + 64], pt[:, 0:64])
                nc.vector.tensor_copy(v_sb[:, base + 65 : base + 129], pt[:, 64:128])

        # ---------------- Phase B: attention ----------------
        # The closeout of q-chunk qc is software-pipelined: the PSUM-freeing
        # eviction (DVE only) runs right after qc's kt-loop, but the PE-using
        # closeout (1/den broadcast, normalize, output projection) is emitted
        # AFTER qc+1's kt-loop so those matmuls never stall the PE stream at
        # the qc boundary (which re-throttled HAM to 1.2 GHz for ~50us each).
        def closeout2(qc, rinvs):
            # Output projection with the softmax normalization folded into
            # the PSUM eviction: separate per-head accumulators Z_h = U_h@Wo_h
            # (U unnormalized), then ob = Z0*r0[s] + Z1*r1[s] with per-
            # partition scalars (r_h = 1/den_h in s-on-partition layout).
            for sti in range(4):
                st = qc * 4 + sti
                r0 = rinvs[0][:, sti : sti + 1]
                r1 = rinvs[1][:, sti : sti + 1]
                for half in (0, 1):
                    z0 = psM.tile([128, 512], F32, tag="m", name="z0")
                    mm(z0, ot0_sb[:, st * 128 : (st + 1) * 128],
                       wo0_sb[:, half * 512 : (half + 1) * 512],
                       start=True, stop=True)
                    z1 = psM.tile([128, 512], F32, tag="m", name="z1")
                    mm(z1, ot1_sb[:, st * 128 : (st + 1) * 128],
                       wo1_sb[:, half * 512 : (half + 1) * 512],
                       start=True, stop=True)
                    ob = outp.tile([128, 512], F32, tag="ob", name="ob")
                    nc.vector.tensor_scalar_mul(ob[:], z0, r0)
                    nc.vector.scalar_tensor_tensor(
                        ob[:], z1, r1, ob[:],
                        op0=mybir.AluOpType.mult, op1=mybir.AluOpType.add)
                    nc.sync.dma_start(
                        out[st * 128 : (st + 1) * 128,
                            half * 512 : (half + 1) * 512],
                        ob[:],
                    )


        prev_closeout = None
        for qc in range(8):
            qspan = slice(qc * 512, (qc + 1) * 512)
            u0 = psU.tile([65, 512], F32, tag="u0", name="u0")
            u1 = psU.tile([65, 512], F32, tag="u1", name="u1")
            for kt in range(32):
                ps = psA.tile([128, 1024], F32, tag="sc", name="ps_s")
                # scores^T [k, q] for both heads (row groups 0:64 / 64:128)
                mm(ps[:, 0:512],
                   kt_sb[0:64, kt * 128 : (kt + 1) * 128],
                   qt_sb[0:64, qspan], start=True, stop=True)
                mm(ps[:, 512:1024],
                   kt_sb[64:128, kt * 128 : (kt + 1) * 128],
                   qt_sb[64:128, qspan], start=True, stop=True)
                ex = exp_pool.tile([128, 1024], BF16, tag="ex", name="ex")
                nc.scalar.activation(ex[:], ps[:], EXP)
                base = kt * 130
                mm(u0, v_sb[:, base : base + 65], ex[:, 0:512],
                   start=(kt == 0), stop=(kt == 31))
                mm(u1, v_sb[:, base + 65 : base + 130], ex[:, 512:1024],
                   start=(kt == 0), stop=(kt == 31))

            # Boundary: evict U PSUM promptly (DVE casts only), and derive
            # 1/den in s-on-partition layout via a tiny linearized DMA
            # reshape [1,512]->[4,128], a small PE transpose to [128,4], and
            # a full-width reciprocal (~60ns, vs 3.3us for the single-lane
            # row reciprocal that used to sit on this boundary).
            rinvs = {}
            for h, u_h, ot_h in ((0, u0, ot0_sb), (1, u1, ot1_sb)):
                nc.vector.tensor_copy(ot_h[:, qspan], u_h[0:64, :])
                den = denp.tile([65, 512], F32R, tag="den", name="den", bufs=4)
                nc.vector.tensor_copy(den[64:65, :], u_h[64:65, :])
                den4 = denp.tile([4, 128], F32R, tag="den4", name="den4", bufs=4)
                nc.sync.dma_start(den4[:, :], den[64:65, :])
                rp = psM.tile([128, 4], F32R, tag="m", name="rp")
                nc.tensor.transpose(rp[:, :], den4[:, :], id_sb[0:4, 0:4])
                rinv = rbp.tile([128, 4], F32, tag="rq", name="rinv", bufs=4)
                nc.vector.reciprocal(rinv[:, :], rp[:, :])
                rinvs[h] = rinv

            if prev_closeout is not None:
                prev_closeout()
            prev_closeout = (lambda q, r: lambda: closeout2(q, r))(qc, rinvs)
        prev_closeout()

    _split_multi_waits(nc)
    return nc


_NC_CACHE: dict = {}


def kernel(x, Wq, bq, Wk, bk, Wv, bv, Wo, bo):
    global LAST_RESULT
    x = np.asarray(x, dtype=np.float32)
    Wq = np.asarray(Wq, dtype=np.float32)
    Wk = np.asarray(Wk, dtype=np.float32)
    Wv = np.asarray(Wv, dtype=np.float32)
    Wo = np.asarray(Wo, dtype=np.float32)
    bq = np.asarray(bq, dtype=np.float32)
    bk = np.asarray(bk, dtype=np.float32)
    bv = np.asarray(bv, dtype=np.float32)
    bo = np.asarray(bo, dtype=np.float32)

    if "nc" not in _NC_CACHE:
        _NC_CACHE["nc"] = _build_nc()
    nc = _NC_CACHE["nc"]

    xT = np.ascontiguousarray(x.reshape(S, DIM).T)
    ident = np.eye(128, dtype=np.float32)
    onesv = np.ones((128, 128), dtype=np.float32)

    in_maps = []
    for c in range(N_CORES):
        sl = slice(c * ICPC, (c + 1) * ICPC)
        in_maps.append({
            "xT": xT,
            "wq": np.ascontiguousarray(Wq[:, sl]),
            "wk": np.ascontiguousarray(Wk[:, sl]),
            "wv": np.ascontiguousarray(Wv[:, sl]),
            "wo0": np.ascontiguousarray(Wo[c * ICPC : c * ICPC + HD, :]),
            "wo1": np.ascontiguousarray(Wo[c * ICPC + HD : (c + 1) * ICPC, :]),
            "bq": np.ascontiguousarray(bq[sl].reshape(ICPC, 1)),
            "bk": np.ascontiguousarray(bk[sl].reshape(ICPC, 1)),
            "ident": ident,
            "onesv": onesv,
        })

    res = run_bass_kernel_spmd(
        nc, in_maps, core_ids=list(range(N_CORES)),
        trace=bool(int(os.environ.get("KERNEL_TRACE", "0"))),
    )
    LAST_RESULT = res

    acc = np.zeros((S, DIM), dtype=np.float64)
    for r in res.results:
        acc += r["out"].astype(np.float64)
    # exact linear corrections handled on host: v-bias and output bias
    acc += (bv.astype(np.float64) @ Wo.astype(np.float64))[None, :]
    acc += bo.astype(np.float64)[None, :]
    return acc.astype(np.float32).reshape(1, S, DIM)


# revision 24
# speedup vs baseline: 1.6346x; 1.0626x over previous
"""Trainium2 Bass kernel for nn_Attention_5042291605734.

Full multi-head attention (B=1, S=4096, DIM=1024, H=16, HD=64), fp32.

Sharding: 2 heads per core (tensor-parallel on inner dim). Each core
computes q/k/v projections for its 128-wide inner slice, full softmax
attention for its 2 heads, and a partial output projection against its
128-row slice of Wo. The host sums the 8 partial outputs and adds the
bias terms (bv @ Wo + bo), which are exactly linear corrections.

Per-core dataflow (all layouts chosen so no on-chip transposes of big
tensors are needed; host passes x pre-transposed):
  xT [DIM, S] (host-transposed)  --mm-->  QT/KT/VT [128, S] (inner on
  partitions).  VT is PE-transposed per 128-tile into V natural [k, hd]
  with an appended ones column (so attn@V also produces the softmax
  denominator as row 64 of the PSUM accumulator).
  Scores are computed in [k, q] layout: ST_tile = KT_slice.T @ QT.
  exp() runs on the Scalar engine PSUM->SBUF; no max-subtraction is
  needed (|scores| <= ~16, exp stays well inside fp32 range).
  U^T[hd(+den), q] += V_aug.T @ exp(ST) accumulated over k tiles.
  Normalization multiplies U^T rows by a PE-broadcast of 1/den.
  Output projection: out[s, :] = O^T[:, s_tile].T @ Wo_c.
"""

import os
from contextlib import ExitStack

import numpy as np

import concourse.bass as bass
import concourse.mybir as mybir
import concourse.tile as tile
from concourse.bass_utils import run_bass_kernel_spmd
from concourse.vector_clock import ScopedClock as _ScopedClock, VectorClock as _VectorClock

_N_PROCS = 27


def _drain_and_barrier_split(self, tick_clock, wait_clock):
    """Replacement for TileContext._drain_and_barrier: the stock version puts
    every live proc's final tick on ONE drain instruction, and this walrus
    build rejects instructions with >2 sync-wait commands (CoreV3GenImpl
    setupSyncWait "Too many sync wait commands"). Split the global-clock wait
    across several sequential drains, one proc each."""
    gc = tick_clock.global_clock
    active = [p for p in range(_N_PROCS) if gc[p] > 0]
    groups = [[p] for p in active] or [[]]
    for grp in groups:
        di = self.nc.sync.drain()
        vc = _VectorClock([gc[p] if p in grp else 0 for p in range(_N_PROCS)])
        wait_clock.add_sem_waits(di.ins, _ScopedClock({None: vc}))
    self.nc.all_engine_barrier()
    assert self.sems is not None
    popped = self.nc._tile_sem_poison_stack.pop()
    assert popped is self._sem_poison
    self.nc.clear_and_free_semaphores(list(self.sems.allocated().values()))
    self.nc.all_engine_barrier()


tile.TileContext._drain_and_barrier = _drain_and_barrier_split


_DMA_INST_TYPES = ("InstDMACopy", "InstDMA", "InstDmaTransposeAnt", "InstDMAGatherAnt",
                   "InstTriggeredCopy", "InstCollectiveCompute")


def _split_multi_waits(nc):
    """This walrus build encodes at most ONE sync-wait command per TPB
    instruction (CoreV3GenImpl setupSyncWait fails with "Too many sync wait
    commands" otherwise). Tile attaches several waits to one instruction, so
    hoist all but one wait onto no-op instructions inserted just before, on
    the same engine. For DMAs this gates the descriptor issue on the issuing
    engine instead of the queue, which is more conservative but correct."""
    for f in nc.m.functions:
        for bb in f.blocks:
            insts = bb.instructions
            out = []
            changed = False
            for inst in insts:
                si = inst.sync_info
                if (si is not None and si.on_wait and len(si.on_wait) > 1
                        and inst.engine != mybir.EngineType.Unassigned):
                    waits = list(si.on_wait)
                    for w in waits[:-1]:
                        out.append(mybir.InstNoOp(
                            name=nc.get_next_instruction_name(),
                            engine=inst.engine,
                            sync_info=mybir.SyncInfo(on_wait=[w], on_update=[]),
                            bass_nofuse=True,
                        ))
                    inst.sync_info = mybir.SyncInfo(
                        on_wait=[waits[-1]], on_update=si.on_update)
                    changed = True
                out.append(inst)
            if changed:
                bb.instructions = out


S = 4096
DIM = 1024
INNER = 1024
H = 16
HD = 64
N_CORES = 8
ICPC = INNER // N_CORES  # inner dims per core = 128

F32 = mybir.dt.float32
F32R = mybir.dt.float32r
BF16 = mybir.dt.bfloat16
EXP = mybir.ActivationFunctionType.Exp

LAST_RESULT = None  # BassKernelResults from the most recent run (for test.py)


def _build_nc() -> bass.Bass:
    nc = bass.Bass()

    xT = nc.declare_dram_parameter("xT", [DIM, S], F32R, isOutput=False)
    wq = nc.declare_dram_parameter("wq", [DIM, ICPC], F32R, isOutput=False)
    wk = nc.declare_dram_parameter("wk", [DIM, ICPC], F32R, isOutput=False)
    wv = nc.declare_dram_parameter("wv", [DIM, ICPC], F32R, isOutput=False)
    wo0 = nc.declare_dram_parameter("wo0", [HD, DIM], F32R, isOutput=False)
    wo1 = nc.declare_dram_parameter("wo1", [HD, DIM], F32R, isOutput=False)
    bq = nc.declare_dram_parameter("bq", [ICPC, 1], F32, isOutput=False)
    bk = nc.declare_dram_parameter("bk", [ICPC, 1], F32, isOutput=False)
    ident = nc.declare_dram_parameter("ident", [128, 128], F32R, isOutput=False)
    onesv = nc.declare_dram_parameter("onesv", [128, 128], F32R, isOutput=False)
    out = nc.declare_dram_parameter("out", [S, DIM], F32, isOutput=True)

    def mm(out_ap, lhsT, rhs, **kw):
        nc.tensor.matmul(out_ap, lhsT, rhs, **kw)

    with tile.TileContext(nc) as tc, ExitStack() as ctx:
        xp = ctx.enter_context(tc.tile_pool(name="xp", bufs=2))
        wp = ctx.enter_context(tc.tile_pool(name="wp", bufs=1))
        bigp = ctx.enter_context(tc.tile_pool(name="bigp", bufs=1))
        vtp = ctx.enter_context(tc.tile_pool(name="vtp", bufs=2))
        exp_pool = ctx.enter_context(tc.tile_pool(name="exp_pool", bufs=4))
        rbp = ctx.enter_context(tc.tile_pool(name="rbp", bufs=2))
        denp = ctx.enter_context(tc.tile_pool(name="denp", bufs=2))
        outp = ctx.enter_context(tc.tile_pool(name="outp", bufs=3))
        psA = ctx.enter_context(tc.tile_pool(name="psA", bufs=2, space="PSUM"))
        psU = ctx.enter_context(tc.tile_pool(name="psU", bufs=1, space="PSUM"))
        psM = ctx.enter_context(tc.tile_pool(name="psM", bufs=2, space="PSUM"))

        # --- weights / constants into SBUF ---
        wq_sb = wp.tile([128, 8 * 128], F32R, tag="wq", name="wq_sb")
        wk_sb = wp.tile([128, 8 * 128], F32R, tag="wk", name="wk_sb")
        wv_sb = wp.tile([128, 8 * 128], F32R, tag="wv", name="wv_sb")
        wo0_sb = wp.tile([64, DIM], F32R, tag="wo0", name="wo0_sb")
        wo1_sb = wp.tile([64, DIM], F32R, tag="wo1", name="wo1_sb")
        bq_sb = wp.tile([128, 1], F32, tag="bq", name="bq_sb")
        bk_sb = wp.tile([128, 1], F32, tag="bk", name="bk_sb")
        id_sb = wp.tile([128, 128], F32R, tag="id", name="id_sb")
        ones_sb = wp.tile([128, 128], F32R, tag="ones", name="ones_sb")

        # [DIM, 128] -> [128, dc*128+m] with element (p, dc*128+m) = W[dc*128+p, m]
        nc.sync.dma_start(wq_sb.rearrange("p (c m) -> p c m", c=8),
                          wq.rearrange("(c p) m -> p c m", p=128))
        nc.sync.dma_start(wk_sb.rearrange("p (c m) -> p c m", c=8),
                          wk.rearrange("(c p) m -> p c m", p=128))
        nc.sync.dma_start(wv_sb.rearrange("p (c m) -> p c m", c=8),
                          wv.rearrange("(c p) m -> p c m", p=128))
        nc.sync.dma_start(wo0_sb[:], wo0[:, :])
        nc.sync.dma_start(wo1_sb[:], wo1[:, :])
        nc.sync.dma_start(bq_sb[:], bq[:, :])
        nc.sync.dma_start(bk_sb[:], bk[:, :])
        nc.sync.dma_start(id_sb[:], ident[:, :])
        nc.sync.dma_start(ones_sb[:], onesv[:, :])

        # --- persistent big SBUF tensors ---
        qt_sb = bigp.tile([128, S], F32R, tag="qt", name="qt_sb")
        kt_sb = bigp.tile([128, S], F32R, tag="kt", name="kt_sb")
        ot0_sb = bigp.tile([64, S], F32R, tag="ot0", name="ot0_sb")
        ot1_sb = bigp.tile([64, S], F32R, tag="ot1", name="ot1_sb")
        # V natural tiles: per k-tile 130 cols = [V_h0(64) | ones | V_h1(64) | ones]
        v_sb = bigp.tile([128, 32 * 130], BF16, tag="v", name="v_sb")

        # ones columns of v_sb (cols 64 + 130*t and 129 + 130*t); memset is
        # ISA-invalid for f32r, so copy from the all-ones constant instead
        for t in range(32):
            nc.vector.tensor_copy(v_sb[:, t * 130 + 64 : t * 130 + 65], ones_sb[:, 0:1])
            nc.vector.tensor_copy(v_sb[:, t * 130 + 129 : t * 130 + 130], ones_sb[:, 0:1])


        def emit_kt(qc, kt, u0, u1):
            qspan = slice(qc * 512, (qc + 1) * 512)
            ps = psA.tile([128, 1024], F32, tag="sc", name="ps_s")
            # scores^T [k, q] for both heads (row groups 0:64 / 64:128)
            mm(ps[:, 0:512],
               kt_sb[0:64, kt * 128 : (kt + 1) * 128],
               qt_sb[0:64, qspan], start=True, stop=True)
            mm(ps[:, 512:1024],
               kt_sb[64:128, kt * 128 : (kt + 1) * 128],
               qt_sb[64:128, qspan], start=True, stop=True)
            ex = exp_pool.tile([128, 1024], BF16, tag="ex", name="ex")
            nc.scalar.activation(ex[:], ps[:], EXP)
            base = kt * 130
            mm(u0, v_sb[:, base : base + 65], ex[:, 0:512],
               start=(kt == 0), stop=(kt == 31))
            mm(u1, v_sb[:, base + 65 : base + 130], ex[:, 512:1024],
               start=(kt == 0), stop=(kt == 31))

        # ---------------- Phase A: projections ----------------
        # For each 512-wide s-span: load xT[:, span] (all 8 dim-chunks),
        # compute QT/KT/VT [128, 512], transpose VT into V natural tiles.
        # q-chunk 0's attention is interleaved per span (its kt tiles become
        # available span by span) so the Scalar engine's exp stream starts
        # ~35us earlier instead of idling through the projections.
        u0_q0 = psU.tile([65, 512], F32, tag="u0", name="u0_q0")
        u1_q0 = psU.tile([65, 512], F32, tag="u1", name="u1_q0")
        for s8 in range(8):
            x8 = xp.tile([128, 8 * 512], F32R, tag="x8", name="x8")
            for dc in range(8):
                nc.sync.dma_start(
                    x8[:, dc * 512 : (dc + 1) * 512],
                    xT[dc * 128 : (dc + 1) * 128, s8 * 512 : (s8 + 1) * 512],
                )
            span = slice(s8 * 512, (s8 + 1) * 512)

            def proj(ps_t, w_sb):
                for dc in range(8):
                    mm(ps_t,
                       w_sb[:, dc * 128 : (dc + 1) * 128],
                       x8[:, dc * 512 : (dc + 1) * 512],
                       start=(dc == 0), stop=(dc == 7))

            ps_q = psA.tile([128, 512], F32, tag="sc", name="ps_q")
            proj(ps_q, wq_sb)
            nc.vector.tensor_scalar_add(qt_sb[:, span], ps_q, bq_sb[:, 0:1])

            ps_k = psA.tile([128, 512], F32, tag="sc", name="ps_k")
            proj(ps_k, wk_sb)
            nc.vector.tensor_scalar_add(kt_sb[:, span], ps_k, bk_sb[:, 0:1])

            ps_v = psA.tile([128, 512], F32, tag="sc", name="ps_v")
            proj(ps_v, wv_sb)
            vt8 = vtp.tile([128, 512], F32R, tag="vt8", name="vt8")
            nc.vector.tensor_copy(vt8[:], ps_v)

            # transpose VT -> V natural [k, inner] per 128-wide k tile
            for kti in range(4):
                kt = s8 * 4 + kti
                pt = psM.tile([128, 128], F32R, tag="m", name="pt")
                nc.tensor.transpose(pt, vt8[:, kti * 128 : (kti + 1) * 128], id_sb[:])
                base = kt * 130
                nc.vector.tensor_copy(v_sb[:, base : base + 64], pt[:, 0:64])
                nc.vector.tensor_copy(v_sb[:, base + 65 : base + 129], pt[:, 64:128])

            for kt in range(4 * s8, 4 * (s8 + 1)):
                emit_kt(0, kt, u0_q0, u1_q0)

        # ---------------- Phase B: attention ----------------
        # The closeout of q-chunk qc is software-pipelined: the PSUM-freeing
        # eviction (DVE only) runs right after qc's kt-loop, but the PE-using
        # closeout (1/den broadcast, normalize, output projection) is emitted
        # AFTER qc+1's kt-loop so those matmuls never stall the PE stream at
        # the qc boundary (which re-throttled HAM to 1.2 GHz for ~50us each).
        def closeout2(qc, rinvs):
            # Output projection with the softmax normalization folded into
            # the PSUM eviction: separate per-head accumulators Z_h = U_h@Wo_h
            # (U unnormalized), then ob = Z0*r0[s] + Z1*r1[s] with per-
            # partition scalars (r_h = 1/den_h in s-on-partition layout).
            for sti in range(4):
                st = qc * 4 + sti
                r0 = rinvs[0][:, sti : sti + 1]
                r1 = rinvs[1][:, sti : sti + 1]
                for half in (0, 1):
                    z0 = psM.tile([128, 512], F32, tag="m", name="z0")
                    mm(z0, ot0_sb[:, st * 128 : (st + 1) * 128],
                       wo0_sb[:, half * 512 : (half + 1) * 512],
                       start=True, stop=True)
                    z1 = psM.tile([128, 512], F32, tag="m", name="z1")
                    mm(z1, ot1_sb[:, st * 128 : (st + 1) * 128],
                       wo1_sb[:, half * 512 : (half + 1) * 512],
                       start=True, stop=True)
                    ob = outp.tile([128, 512], F32, tag="ob", name="ob")
                    nc.vector.tensor_scalar_mul(ob[:], z0, r0)
                    nc.vector.scalar_tensor_tensor(
                        ob[:], z1, r1, ob[:],
                        op0=mybir.AluOpType.mult, op1=mybir.AluOpType.add)
                    nc.sync.dma_start(
                        out[st * 128 : (st + 1) * 128,
                            half * 512 : (half + 1) * 512],
                        ob[:],
                    )


        prev_closeout = None
        for qc in range(8):
            qspan = slice(qc * 512, (qc + 1) * 512)
            if qc == 0:
                u0, u1 = u0_q0, u1_q0
            else:
                u0 = psU.tile([65, 512], F32, tag="u0", name="u0")
                u1 = psU.tile([65, 512], F32, tag="u1", name="u1")
                for kt in range(32):
                    emit_kt(qc, kt, u0, u1)

            # Boundary: evict U PSUM promptly (DVE casts only), and derive
            # 1/den in s-on-partition layout via a tiny linearized DMA
            # reshape [1,512]->[4,128], a small PE transpose to [128,4], and
            # a full-width reciprocal (~60ns, vs 3.3us for the single-lane
            # row reciprocal that used to sit on this boundary).
            rinvs = {}
            for h, u_h, ot_h in ((0, u0, ot0_sb), (1, u1, ot1_sb)):
                nc.vector.tensor_copy(ot_h[:, qspan], u_h[0:64, :])
                den = denp.tile([65, 512], F32R, tag="den", name="den", bufs=4)
                nc.vector.tensor_copy(den[64:65, :], u_h[64:65, :])
                den4 = denp.tile([4, 128], F32R, tag="den4", name="den4", bufs=4)
                nc.sync.dma_start(den4[:, :], den[64:65, :])
                rp = psM.tile([128, 4], F32R, tag="m", name="rp")
                nc.tensor.transpose(rp[:, :], den4[:, :], id_sb[0:4, 0:4])
                rinv = rbp.tile([128, 4], F32, tag="rq", name="rinv", bufs=4)
                nc.vector.reciprocal(rinv[:, :], rp[:, :])
                rinvs[h] = rinv

            if prev_closeout is not None:
                prev_closeout()
            prev_closeout = (lambda q, r: lambda: closeout2(q, r))(qc, rinvs)
        prev_closeout()

    _split_multi_waits(nc)
    return nc


_NC_CACHE: dict = {}


def kernel(x, Wq, bq, Wk, bk, Wv, bv, Wo, bo):
    global LAST_RESULT
    x = np.asarray(x, dtype=np.float32)
    Wq = np.asarray(Wq, dtype=np.float32)
    Wk = np.asarray(Wk, dtype=np.float32)
    Wv = np.asarray(Wv, dtype=np.float32)
    Wo = np.asarray(Wo, dtype=np.float32)
    bq = np.asarray(bq, dtype=np.float32)
    bk = np.asarray(bk, dtype=np.float32)
    bv = np.asarray(bv, dtype=np.float32)
    bo = np.asarray(bo, dtype=np.float32)

    if "nc" not in _NC_CACHE:
        _NC_CACHE["nc"] = _build_nc()
    nc = _NC_CACHE["nc"]

    xT = np.ascontiguousarray(x.reshape(S, DIM).T)
    ident = np.eye(128, dtype=np.float32)
    onesv = np.ones((128, 128), dtype=np.float32)

    in_maps = []
    for c in range(N_CORES):
        sl = slice(c * ICPC, (c + 1) * ICPC)
        in_maps.append({
            "xT": xT,
            "wq": np.ascontiguousarray(Wq[:, sl]),
            "wk": np.ascontiguousarray(Wk[:, sl]),
            "wv": np.ascontiguousarray(Wv[:, sl]),
            "wo0": np.ascontiguousarray(Wo[c * ICPC : c * ICPC + HD, :]),
            "wo1": np.ascontiguousarray(Wo[c * ICPC + HD : (c + 1) * ICPC, :]),
            "bq": np.ascontiguousarray(bq[sl].reshape(ICPC, 1)),
            "bk": np.ascontiguousarray(bk[sl].reshape(ICPC, 1)),
            "ident": ident,
            "onesv": onesv,
        })

    res = run_bass_kernel_spmd(
        nc, in_maps, core_ids=list(range(N_CORES)),
        trace=bool(int(os.environ.get("KERNEL_TRACE", "0"))),
    )
    LAST_RESULT = res

    acc = np.zeros((S, DIM), dtype=np.float64)
    for r in res.results:
        acc += r["out"].astype(np.float64)
    # exact linear corrections handled on host: v-bias and output bias
    acc += (bv.astype(np.float64) @ Wo.astype(np.float64))[None, :]
    acc += bo.astype(np.float64)[None, :]
    return acc.astype(np.float32).reshape(1, S, DIM)
